# revision 15
# baseline (speedup 1.0000x reference)
"""Trainium2 Bass kernel for nn_Attention_1898375545286 (sparse/triangle attention).

Per pair-row n (256 of them, 32 per core x 8 cores):
  q = (q_x[n] @ Wq)/sqrt(32), k = kv_x[n] @ Wk, v = kv_x[n] @ Wv  (heads of 32)
  a = softmax_k(q.k + mask_bias[n,k] + tri_bias[h,q,k])
  out[n] = ((a @ v) * sigmoid(q_x[n] @ Wg)) @ Wo

Device dataflow (everything "transposed": hc/c on partitions, q on free):
  - host pre-transposes q_x/kv_x to [n, c, q] so projections need no on-chip transpose
  - attention computed as a^T [k, q]: QK via row-tiled (K=32) packed matmuls,
    tri_bias pre-accumulated into PSUM via identity-stationary f32r matmuls,
    mask_bias applied as the per-partition bias of the ACT exp
  - softmax denominator via column-tiled ones-matmul that also broadcasts the
    per-head sum across the head's 32 partitions; normalization folded into the
    sigmoid gate: o * sigmoid(g) / s == o / (s * (1 + exp(-g)))
  - output projection keeps q on partitions so the result DMAs out contiguously
Matmul dtypes: f32r for DMA-fed operands (projections, bias writes), bf16 for
the feedback path (QK, sums, AV, out-proj) — fp32 PSUM accumulation throughout.
"""
import sys

sys.path.insert(0, "/opt/trn_rl_repo")

import math

import numpy as np
import ml_dtypes

N_CORES = 8
B, N, Q, C = 1, 256, 256, 128
H, C_HID = 4, 32
ROWS = N // N_CORES  # rows per core

_cache = {}


def _build(mask_zero=True):
    import concourse.bass as bass
    import concourse.tile as tile
    from concourse import mybir, bacc

    f32 = mybir.dt.float32
    f32r = mybir.dt.float32r
    bf16 = mybir.dt.bfloat16
    Exp = mybir.ActivationFunctionType.Exp

    nc = bacc.Bacc("TRN2", target_bir_lowering=False, debug=False,
                   num_devices=N_CORES)

    qxT = nc.dram_tensor("qxT", [ROWS, C, Q], f32r, kind="ExternalInput").ap()
    kxT = nc.dram_tensor("kxT", [ROWS, C, Q], f32r, kind="ExternalInput").ap()
    tri = nc.dram_tensor("tri", [128, 2 * H * Q], f32r, kind="ExternalInput").ap()
    wq = nc.dram_tensor("wq", [C, C], f32r, kind="ExternalInput").ap()
    wk = nc.dram_tensor("wk", [C, C], f32r, kind="ExternalInput").ap()
    wv = nc.dram_tensor("wv", [C, C], f32r, kind="ExternalInput").ap()
    wg = nc.dram_tensor("wg", [C, C], f32r, kind="ExternalInput").ap()
    wo = nc.dram_tensor("wo", [C, C], bf16, kind="ExternalInput").ap()
    eye = nc.dram_tensor("eye", [C, C], f32r, kind="ExternalInput").ap()
    ones32 = nc.dram_tensor("ones32", [128, 32], bf16, kind="ExternalInput").ap()
    if not mask_zero:
        maskd = nc.dram_tensor("maskd", [128, ROWS, 2], f32,
                               kind="ExternalInput").ap()
    out_d = nc.dram_tensor("out", [ROWS, Q, C], f32, kind="ExternalOutput").ap()

    with tile.TileContext(nc) as tc:
        with tc.tile_pool(name="const", bufs=1) as cpool, \
             tc.tile_pool(name="xin", bufs=3) as xpool, \
             tc.tile_pool(name="projsb", bufs=2) as ppool, \
             tc.tile_pool(name="aexp", bufs=2) as epool, \
             tc.tile_pool(name="gate", bufs=2) as gpool, \
             tc.tile_pool(name="proj_ps", bufs=1, space="PSUM") as proj_pool, \
             tc.tile_pool(name="at_ps", bufs=1, space="PSUM") as at_pool, \
             tc.tile_pool(name="so_ps", bufs=1, space="PSUM") as so_pool:

            tri_sb = cpool.tile([128, 2 * H * Q], f32r)
            wq_sb = cpool.tile([C, C], f32r, tag="wq")
            wk_sb = cpool.tile([C, C], f32r, tag="wk")
            wv_sb = cpool.tile([C, C], f32r, tag="wv")
            wg_sb = cpool.tile([C, C], f32r, tag="wg")
            wo_sb = cpool.tile([C, C], bf16, tag="wo")
            eye_sb = cpool.tile([C, C], f32r, tag="eye")
            ones_sb = cpool.tile([128, 32], bf16, tag="ones")
            nc.sync.dma_start(out=tri_sb[:], in_=tri[:])
            nc.sync.dma_start(out=wq_sb[:], in_=wq[:])
            nc.sync.dma_start(out=wk_sb[:], in_=wk[:])
            nc.sync.dma_start(out=wv_sb[:], in_=wv[:])
            nc.sync.dma_start(out=wg_sb[:], in_=wg[:])
            nc.sync.dma_start(out=wo_sb[:], in_=wo[:])
            nc.sync.dma_start(out=eye_sb[:], in_=eye[:])
            nc.sync.dma_start(out=ones_sb[:], in_=ones32[:])
            if not mask_zero:
                mask_sb = cpool.tile([128, ROWS, 2], f32, tag="mask")
                nc.sync.dma_start(out=mask_sb[:], in_=maskd[:])

            for n in range(ROWS):
                qx_sb = xpool.tile([C, Q], f32r, tag="qx")
                kx_sb = xpool.tile([C, Q], f32r, tag="kx")
                nc.sync.dma_start(out=qx_sb[:], in_=qxT[n])
                nc.sync.dma_start(out=kx_sb[:], in_=kxT[n])

                # proj mega psum tile: qT 0:256 | kT 256:512 | gT 512:768 |
                # v 768:1024 | oT 1024:1280   (3 banks)
                pp = proj_pool.tile([128, 1536], f32, tag="pp")
                qT_ps = pp[:, 0:256]
                kT_ps = pp[:, 256:512]
                gT_ps = pp[:, 512:768]
                v_ps = pp[:, 768:1024]
                oT_ps = pp[:, 1024:1280]

                # bank0 group: qT then kT; bank1 group: gT then v(kc0,kc1)
                nc.tensor.matmul(qT_ps, lhsT=wq_sb[:], rhs=qx_sb[:],
                                 start=True, stop=False, skip_group_check=True)
                nc.tensor.matmul(kT_ps, lhsT=wk_sb[:], rhs=kx_sb[:],
                                 start=False, stop=True, skip_group_check=True)
                nc.tensor.matmul(gT_ps, lhsT=wg_sb[:], rhs=qx_sb[:],
                                 start=True, stop=False, skip_group_check=True)
                for kc in range(2):
                    nc.tensor.matmul(v_ps[:, kc * 128:(kc + 1) * 128],
                                     lhsT=kx_sb[:, kc * 128:(kc + 1) * 128],
                                     rhs=wv_sb[:], start=False, stop=(kc == 1), skip_group_check=True)

                # aT mega tile [128, (kc, h, q)] (4 banks); tri_bias written
                # first (start=True), QK accumulates on top
                # each 2KB bank holds two 256-col slices (h even/odd pair):
                # one group per bank spanning [bias even, bias odd, QK even, QK odd]
                at = at_pool.tile([128, 2 * H * Q], f32, tag="at")
                for h in range(H):
                    for kc in range(2):
                        s = (h * 2 + kc) * Q
                        nc.tensor.matmul(at[:, s:s + Q], lhsT=eye_sb[:],
                                         rhs=tri_sb[:, s:s + Q],
                                         start=(kc == 0), stop=False, skip_group_check=True)

                # evacuate projections (bf16 for the feedback matmuls)
                qT_sb = ppool.tile([C, Q], bf16, tag="qT")
                kT_sb = ppool.tile([C, Q], bf16, tag="kT")
                v_sb = ppool.tile([128, 256], bf16, tag="v")
                nc.vector.tensor_copy(out=qT_sb[:], in_=qT_ps)
                nc.vector.tensor_copy(out=kT_sb[:], in_=kT_ps)
                nc.vector.tensor_copy(out=v_sb[:], in_=v_ps)
                # exp(-g) for the sigmoid-gate fold (same ACT table as Exp)
                ag_sb = ppool.tile([C, Q], f32, tag="ag")
                nc.scalar.activation(ag_sb[:], gT_ps, Exp, scale=-1.0)

                # QK: row-tiled K=32 packed matmuls, accumulate onto tri bias
                for kc in range(2):
                    for h in range(H):
                        s = (h * 2 + kc) * Q
                        nc.tensor.matmul(
                            at[:, s:s + Q],
                            lhsT=kT_sb[32 * h:32 * (h + 1), kc * 128:(kc + 1) * 128],
                            rhs=qT_sb[32 * h:32 * (h + 1), :],
                            start=False, stop=(kc == 1),
                            tile_position=(32 * h, 0),
                            skip_group_check=True)

                # softmax numerator: exp over the whole mega tile
                aexp_sb = epool.tile([128, 2 * H * Q], bf16, tag="aexp")
                if mask_zero:
                    nc.scalar.activation(aexp_sb[:], at[:], Exp)
                else:
                    atv = at[:].rearrange("p (h k q) -> p h k q", h=H, k=2)
                    aev = aexp_sb[:].rearrange("p (h k q) -> p h k q", h=H, k=2)
                    for kc in range(2):
                        nc.scalar.activation(aev[:, :, kc, :], atv[:, :, kc, :],
                                             Exp, bias=mask_sb[:, n, kc])

                # denominator, broadcast across each head's 32 partitions via
                # all-ones stationary; col-tiled packed
                so_t = so_pool.tile([128, 512], f32, tag="so")
                so = so_t[:, 0:256]
                for kc in range(2):
                    for h in range(H):
                        s = (h * 2 + kc) * Q
                        nc.tensor.matmul(so[32 * h:32 * (h + 1), :],
                                         lhsT=ones_sb[:],
                                         rhs=aexp_sb[:, s:s + Q],
                                         start=(kc == 0), stop=(kc == 1),
                                         tile_position=(0, 32 * h),
                                         skip_group_check=True)
                # attention values: col-tiled packed, accumulate over kc
                for kc in range(2):
                    for h in range(H):
                        s = (h * 2 + kc) * Q
                        nc.tensor.matmul(
                            pp[32 * h:32 * (h + 1), 1024:1280],
                            lhsT=v_sb[:, kc * 128 + 32 * h:kc * 128 + 32 * (h + 1)],
                            rhs=aexp_sb[:, s:s + Q],
                            start=(kc == 0), stop=(kc == 1),
                            tile_position=(0, 32 * h),
                            skip_group_check=True)

                # gate+normalize: ge = 1/(s*(1+exp(-g))); o_f = oT * ge
                u2 = gpool.tile([C, Q], f32, tag="u2")
                ge = gpool.tile([C, Q], f32, tag="ge")
                of_sb = gpool.tile([C, Q], bf16, tag="of")
                nc.vector.scalar_tensor_tensor(
                    out=u2[:], in0=ag_sb[:], scalar=1.0, in1=so[:],
                    op0=mybir.AluOpType.add, op1=mybir.AluOpType.mult)
                nc.vector.reciprocal_approx_fast(out=ge[:], in_=u2[:])
                nc.vector.tensor_tensor(out=of_sb[:], in0=oT_ps, in1=ge[:],
                                        op=mybir.AluOpType.mult)

                # output projection: q on partitions, c on free -> direct DMA
                op_t = so_pool.tile([128, 512], f32, tag="so")
                op = op_t[:, 0:256]
                for qc in range(2):
                    nc.tensor.matmul(op[:, qc * 128:(qc + 1) * 128],
                                     lhsT=of_sb[:, qc * 128:(qc + 1) * 128],
                                     rhs=wo_sb[:], start=(qc == 0), stop=(qc == 1), skip_group_check=True)
                out_sb = gpool.tile([128, 256], f32, tag="osb")
                nc.vector.tensor_copy(out=out_sb[:], in_=op[:])
                for qc in range(2):
                    nc.sync.dma_start(out=out_d[n, qc * 128:(qc + 1) * 128, :],
                                      in_=out_sb[:, qc * 128:(qc + 1) * 128])
    nc.compile()
    return nc


def _host_prep(inputs):
    q_x = np.ascontiguousarray(inputs["q_x"], np.float32)[0]    # [N, Q, C]
    kv_x = np.ascontiguousarray(inputs["kv_x"], np.float32)[0]
    tri_b = np.asarray(inputs["tri_bias"], np.float32)[0, 0]    # [H, Q, K]
    mask_b = np.asarray(inputs["mask_bias"], np.float32)[0, :, 0, 0, :]  # [N, K]
    Wq = np.asarray(inputs["Wq"], np.float32) / math.sqrt(C_HID)
    Wk = np.asarray(inputs["Wk"], np.float32)
    Wv = np.asarray(inputs["Wv"], np.float32)
    Wg = np.asarray(inputs["Wg"], np.float32)
    Wo = np.asarray(inputs["Wo"], np.float32)

    qxT = np.ascontiguousarray(q_x.transpose(0, 2, 1))   # [N, C, Q]
    kxT = np.ascontiguousarray(kv_x.transpose(0, 2, 1))

    # tri layout: [128, (h, kc, q)]; tri[p, (h*2+kc)*Q + q] = tri_b[h, q, kc*128+p]
    tri_dev = np.empty((128, 2 * H * Q), np.float32)
    for h in range(H):
        for kc in range(2):
            s = (h * 2 + kc) * Q
            tri_dev[:, s:s + Q] = tri_b[h, :, kc * 128:(kc + 1) * 128].T

    shared = {
        "tri": tri_dev,
        "wq": Wq, "wk": Wk, "wv": Wv, "wg": Wg,
        "wo": Wo.astype(ml_dtypes.bfloat16),
        "eye": np.eye(C, dtype=np.float32),
        "ones32": np.ones((128, 32), ml_dtypes.bfloat16),
    }
    in_maps = []
    for c in range(N_CORES):
        r0 = c * ROWS
        in_maps.append({
            "qxT": np.ascontiguousarray(qxT[r0:r0 + ROWS]),
            "kxT": np.ascontiguousarray(kxT[r0:r0 + ROWS]),
            **shared,
        })
    return in_maps, mask_b


def kernel(**inputs):
    from concourse import bass_utils

    in_maps, mask_b = _host_prep(inputs)
    mask_zero = bool(np.all(mask_b == 0.0))
    if not mask_zero:
        # mask layout [128, rows, kc]: mask[p, n, kc] = mask_b[row, kc*128+p]
        for c in range(N_CORES):
            r0 = c * ROWS
            md = np.empty((128, ROWS, 2), np.float32)
            for kc in range(2):
                md[:, :, kc] = mask_b[r0:r0 + ROWS, kc * 128:(kc + 1) * 128].T
            in_maps[c]["maskd"] = md
    key = ("nc", mask_zero)
    if key not in _cache:
        _cache[key] = _build(mask_zero)
    nc = _cache[key]
    res = bass_utils.run_bass_kernel_spmd(nc, in_maps, list(range(N_CORES)))
    out = np.concatenate([res.results[c]["out"] for c in range(N_CORES)], axis=0)
    return out.reshape(B, N, Q, C)


# revision 16
# speedup vs baseline: 19.5403x; 19.5403x over previous
"""Trainium2 Bass kernel for nn_Attention_1898375545286 (sparse/triangle attention).

Per pair-row n (256 of them, 32 per core x 8 cores):
  q = (q_x[n] @ Wq)/sqrt(32), k = kv_x[n] @ Wk, v = kv_x[n] @ Wv  (heads of 32)
  a = softmax_k(q.k + mask_bias[n,k] + tri_bias[h,q,k])
  out[n] = ((a @ v) * sigmoid(q_x[n] @ Wg)) @ Wo

Device dataflow (everything "transposed": hc/c on partitions, q on free):
  - host pre-transposes q_x/kv_x to [n, c, q] so projections need no on-chip transpose
  - attention computed as a^T [k, q]: QK via row-tiled (K=32) packed matmuls,
    tri_bias pre-accumulated into PSUM via identity-stationary f32r matmuls,
    mask_bias applied as the per-partition bias of the ACT exp
  - softmax denominator via column-tiled ones-matmul that also broadcasts the
    per-head sum across the head's 32 partitions; normalization folded into the
    sigmoid gate: o * sigmoid(g) / s == o / (s * (1 + exp(-g)))
  - output projection keeps q on partitions so the result DMAs out contiguously
Matmul dtypes: f32r for DMA-fed operands (projections, bias writes), bf16 for
the feedback path (QK, sums, AV, out-proj) — fp32 PSUM accumulation throughout.
"""
import sys

sys.path.insert(0, "/opt/trn_rl_repo")

import math

import numpy as np
import ml_dtypes

N_CORES = 8
B, N, Q, C = 1, 256, 256, 128
H, C_HID = 4, 32
ROWS = N // N_CORES  # rows per core

_cache = {}


def _build(mask_zero=True, repeats=1):
    import concourse.bass as bass
    import concourse.tile as tile
    from concourse import mybir, bacc

    f32 = mybir.dt.float32
    f32r = mybir.dt.float32r
    bf16 = mybir.dt.bfloat16
    Exp = mybir.ActivationFunctionType.Exp

    nc = bacc.Bacc("TRN2", target_bir_lowering=False, debug=False,
                   num_devices=N_CORES)

    qxT = nc.dram_tensor("qxT", [ROWS, C, Q], f32r, kind="ExternalInput").ap()
    kxT = nc.dram_tensor("kxT", [ROWS, C, Q], f32r, kind="ExternalInput").ap()
    tri = nc.dram_tensor("tri", [128, 2 * H * Q], f32r, kind="ExternalInput").ap()
    wq = nc.dram_tensor("wq", [C, C], f32r, kind="ExternalInput").ap()
    wk = nc.dram_tensor("wk", [C, C], f32r, kind="ExternalInput").ap()
    wv = nc.dram_tensor("wv", [C, C], f32r, kind="ExternalInput").ap()
    wg = nc.dram_tensor("wg", [C, C], f32r, kind="ExternalInput").ap()
    wo = nc.dram_tensor("wo", [C, C], bf16, kind="ExternalInput").ap()
    eye = nc.dram_tensor("eye", [C, C], f32r, kind="ExternalInput").ap()
    ones32 = nc.dram_tensor("ones32", [128, 32], bf16, kind="ExternalInput").ap()
    if not mask_zero:
        maskd = nc.dram_tensor("maskd", [128, ROWS, 2], f32,
                               kind="ExternalInput").ap()
    out_d = nc.dram_tensor("out", [ROWS, Q, C], f32, kind="ExternalOutput").ap()

    with tile.TileContext(nc) as tc:
        with tc.tile_pool(name="const", bufs=1) as cpool, \
             tc.tile_pool(name="xin", bufs=3) as xpool, \
             tc.tile_pool(name="projsb", bufs=2) as ppool, \
             tc.tile_pool(name="aexp", bufs=2) as epool, \
             tc.tile_pool(name="gate", bufs=2) as gpool, \
             tc.tile_pool(name="proj_ps", bufs=1, space="PSUM") as proj_pool, \
             tc.tile_pool(name="at_ps", bufs=1, space="PSUM") as at_pool, \
             tc.tile_pool(name="so_ps", bufs=1, space="PSUM") as so_pool:

            tri_sb = cpool.tile([128, 2 * H * Q], f32r)
            wq_sb = cpool.tile([C, C], f32r, tag="wq")
            wk_sb = cpool.tile([C, C], f32r, tag="wk")
            wv_sb = cpool.tile([C, C], f32r, tag="wv")
            wg_sb = cpool.tile([C, C], f32r, tag="wg")
            wo_sb = cpool.tile([C, C], bf16, tag="wo")
            eye_sb = cpool.tile([C, C], f32r, tag="eye")
            ones_sb = cpool.tile([128, 32], bf16, tag="ones")
            nc.sync.dma_start(out=tri_sb[:], in_=tri[:])
            nc.sync.dma_start(out=wq_sb[:], in_=wq[:])
            nc.sync.dma_start(out=wk_sb[:], in_=wk[:])
            nc.sync.dma_start(out=wv_sb[:], in_=wv[:])
            nc.sync.dma_start(out=wg_sb[:], in_=wg[:])
            nc.sync.dma_start(out=wo_sb[:], in_=wo[:])
            nc.sync.dma_start(out=eye_sb[:], in_=eye[:])
            nc.sync.dma_start(out=ones_sb[:], in_=ones32[:])
            if not mask_zero:
                mask_sb = cpool.tile([128, ROWS, 2], f32, tag="mask")
                nc.sync.dma_start(out=mask_sb[:], in_=maskd[:])

            for _rep in range(repeats):
              for n in range(ROWS):
                qx_sb = xpool.tile([C, Q], f32r, tag="qx")
                kx_sb = xpool.tile([C, Q], f32r, tag="kx")
                nc.sync.dma_start(out=qx_sb[:], in_=qxT[n])
                nc.sync.dma_start(out=kx_sb[:], in_=kxT[n])

                # proj mega psum tile: qT 0:256 | kT 256:512 | gT 512:768 |
                # v 768:1024 | oT 1024:1280   (3 banks)
                pp = proj_pool.tile([128, 1536], f32, tag="pp")
                qT_ps = pp[:, 0:256]
                kT_ps = pp[:, 256:512]
                gT_ps = pp[:, 512:768]
                v_ps = pp[:, 768:1024]
                oT_ps = pp[:, 1024:1280]

                # bank0 group: qT then kT; bank1 group: gT then v(kc0,kc1)
                nc.tensor.matmul(qT_ps, lhsT=wq_sb[:], rhs=qx_sb[:],
                                 start=True, stop=False, skip_group_check=True)
                nc.tensor.matmul(kT_ps, lhsT=wk_sb[:], rhs=kx_sb[:],
                                 start=False, stop=True, skip_group_check=True)
                nc.tensor.matmul(gT_ps, lhsT=wg_sb[:], rhs=qx_sb[:],
                                 start=True, stop=False, skip_group_check=True)
                for kc in range(2):
                    nc.tensor.matmul(v_ps[:, kc * 128:(kc + 1) * 128],
                                     lhsT=kx_sb[:, kc * 128:(kc + 1) * 128],
                                     rhs=wv_sb[:], start=False, stop=(kc == 1), skip_group_check=True)

                # aT mega tile [128, (kc, h, q)] (4 banks); tri_bias written
                # first (start=True), QK accumulates on top
                # each 2KB bank holds two 256-col slices (h even/odd pair):
                # one group per bank spanning [bias even, bias odd, QK even, QK odd]
                at = at_pool.tile([128, 2 * H * Q], f32, tag="at")
                for h in range(H):
                    for kc in range(2):
                        s = (h * 2 + kc) * Q
                        nc.tensor.matmul(at[:, s:s + Q], lhsT=eye_sb[:],
                                         rhs=tri_sb[:, s:s + Q],
                                         start=(kc == 0), stop=False, skip_group_check=True)

                # evacuate projections (bf16 for the feedback matmuls)
                qT_sb = ppool.tile([C, Q], bf16, tag="qT")
                kT_sb = ppool.tile([C, Q], bf16, tag="kT")
                v_sb = ppool.tile([128, 256], bf16, tag="v")
                nc.vector.tensor_copy(out=qT_sb[:], in_=qT_ps)
                nc.vector.tensor_copy(out=kT_sb[:], in_=kT_ps)
                nc.vector.tensor_copy(out=v_sb[:], in_=v_ps)
                # exp(-g) for the sigmoid-gate fold (same ACT table as Exp)
                ag_sb = ppool.tile([C, Q], f32, tag="ag")
                nc.scalar.activation(ag_sb[:], gT_ps, Exp, scale=-1.0)

                # QK: row-tiled K=32 packed matmuls, accumulate onto tri bias
                for kc in range(2):
                    for h in range(H):
                        s = (h * 2 + kc) * Q
                        nc.tensor.matmul(
                            at[:, s:s + Q],
                            lhsT=kT_sb[32 * h:32 * (h + 1), kc * 128:(kc + 1) * 128],
                            rhs=qT_sb[32 * h:32 * (h + 1), :],
                            start=False, stop=(kc == 1),
                            tile_position=(32 * h, 0),
                            skip_group_check=True)

                # softmax numerator: exp over the whole mega tile
                aexp_sb = epool.tile([128, 2 * H * Q], bf16, tag="aexp")
                if mask_zero:
                    nc.scalar.activation(aexp_sb[:], at[:], Exp)
                else:
                    atv = at[:].rearrange("p (h k q) -> p h k q", h=H, k=2)
                    aev = aexp_sb[:].rearrange("p (h k q) -> p h k q", h=H, k=2)
                    for kc in range(2):
                        nc.scalar.activation(aev[:, :, kc, :], atv[:, :, kc, :],
                                             Exp, bias=mask_sb[:, n, kc])

                # denominator, broadcast across each head's 32 partitions via
                # all-ones stationary; col-tiled packed
                so_t = so_pool.tile([128, 512], f32, tag="so")
                so = so_t[:, 0:256]
                for kc in range(2):
                    for h in range(H):
                        s = (h * 2 + kc) * Q
                        nc.tensor.matmul(so[32 * h:32 * (h + 1), :],
                                         lhsT=ones_sb[:],
                                         rhs=aexp_sb[:, s:s + Q],
                                         start=(kc == 0), stop=(kc == 1),
                                         tile_position=(0, 32 * h),
                                         skip_group_check=True)
                # attention values: col-tiled packed, accumulate over kc
                for kc in range(2):
                    for h in range(H):
                        s = (h * 2 + kc) * Q
                        nc.tensor.matmul(
                            pp[32 * h:32 * (h + 1), 1024:1280],
                            lhsT=v_sb[:, kc * 128 + 32 * h:kc * 128 + 32 * (h + 1)],
                            rhs=aexp_sb[:, s:s + Q],
                            start=(kc == 0), stop=(kc == 1),
                            tile_position=(0, 32 * h),
                            skip_group_check=True)

                # gate+normalize: ge = 1/(s*(1+exp(-g))); o_f = oT * ge
                u2 = gpool.tile([C, Q], f32, tag="u2")
                ge = gpool.tile([C, Q], f32, tag="ge")
                of_sb = gpool.tile([C, Q], bf16, tag="of")
                nc.vector.scalar_tensor_tensor(
                    out=u2[:], in0=ag_sb[:], scalar=1.0, in1=so[:],
                    op0=mybir.AluOpType.add, op1=mybir.AluOpType.mult)
                nc.vector.reciprocal_approx_fast(out=ge[:], in_=u2[:])
                nc.vector.tensor_tensor(out=of_sb[:], in0=oT_ps, in1=ge[:],
                                        op=mybir.AluOpType.mult)

                # output projection: q on partitions, c on free -> direct DMA
                op_t = so_pool.tile([128, 512], f32, tag="so")
                op = op_t[:, 0:256]
                for qc in range(2):
                    nc.tensor.matmul(op[:, qc * 128:(qc + 1) * 128],
                                     lhsT=of_sb[:, qc * 128:(qc + 1) * 128],
                                     rhs=wo_sb[:], start=(qc == 0), stop=(qc == 1), skip_group_check=True)
                out_sb = gpool.tile([128, 256], f32, tag="osb")
                nc.vector.tensor_copy(out=out_sb[:], in_=op[:])
                for qc in range(2):
                    nc.sync.dma_start(out=out_d[n, qc * 128:(qc + 1) * 128, :],
                                      in_=out_sb[:, qc * 128:(qc + 1) * 128])
    nc.compile()
    return nc


def _host_prep(inputs):
    q_x = np.ascontiguousarray(inputs["q_x"], np.float32)[0]    # [N, Q, C]
    kv_x = np.ascontiguousarray(inputs["kv_x"], np.float32)[0]
    tri_b = np.asarray(inputs["tri_bias"], np.float32)[0, 0]    # [H, Q, K]
    mask_b = np.asarray(inputs["mask_bias"], np.float32)[0, :, 0, 0, :]  # [N, K]
    Wq = np.asarray(inputs["Wq"], np.float32) / math.sqrt(C_HID)
    Wk = np.asarray(inputs["Wk"], np.float32)
    Wv = np.asarray(inputs["Wv"], np.float32)
    Wg = np.asarray(inputs["Wg"], np.float32)
    Wo = np.asarray(inputs["Wo"], np.float32)

    qxT = np.ascontiguousarray(q_x.transpose(0, 2, 1))   # [N, C, Q]
    kxT = np.ascontiguousarray(kv_x.transpose(0, 2, 1))

    # tri layout: [128, (h, kc, q)]; tri[p, (h*2+kc)*Q + q] = tri_b[h, q, kc*128+p]
    tri_dev = np.empty((128, 2 * H * Q), np.float32)
    for h in range(H):
        for kc in range(2):
            s = (h * 2 + kc) * Q
            tri_dev[:, s:s + Q] = tri_b[h, :, kc * 128:(kc + 1) * 128].T

    shared = {
        "tri": tri_dev,
        "wq": Wq, "wk": Wk, "wv": Wv, "wg": Wg,
        "wo": Wo.astype(ml_dtypes.bfloat16),
        "eye": np.eye(C, dtype=np.float32),
        "ones32": np.ones((128, 32), ml_dtypes.bfloat16),
    }
    in_maps = []
    for c in range(N_CORES):
        r0 = c * ROWS
        in_maps.append({
            "qxT": np.ascontiguousarray(qxT[r0:r0 + ROWS]),
            "kxT": np.ascontiguousarray(kxT[r0:r0 + ROWS]),
            **shared,
        })
    return in_maps, mask_b


def kernel(**inputs):
    from concourse import bass_utils

    in_maps, mask_b = _host_prep(inputs)
    mask_zero = bool(np.all(mask_b == 0.0))
    if not mask_zero:
        # mask layout [128, rows, kc]: mask[p, n, kc] = mask_b[row, kc*128+p]
        for c in range(N_CORES):
            r0 = c * ROWS
            md = np.empty((128, ROWS, 2), np.float32)
            for kc in range(2):
                md[:, :, kc] = mask_b[r0:r0 + ROWS, kc * 128:(kc + 1) * 128].T
            in_maps[c]["maskd"] = md
    key = ("nc", mask_zero)
    if key not in _cache:
        _cache[key] = _build(mask_zero)
    nc = _cache[key]
    res = bass_utils.run_bass_kernel_spmd(nc, in_maps, list(range(N_CORES)))
    out = np.concatenate([res.results[c]["out"] for c in range(N_CORES)], axis=0)
    return out.reshape(B, N, Q, C)


# revision 18
# speedup vs baseline: 20.3299x; 1.0404x over previous
"""Trainium2 Bass kernel for nn_Attention_1898375545286 (sparse/triangle attention).

Per pair-row n (256 of them, 32 per core x 8 cores):
  q = (q_x[n] @ Wq)/sqrt(32), k = kv_x[n] @ Wk, v = kv_x[n] @ Wv  (heads of 32)
  a = softmax_k(q.k + mask_bias[n,k] + tri_bias[h,q,k])
  out[n] = ((a @ v) * sigmoid(q_x[n] @ Wg)) @ Wo

Device dataflow (everything "transposed": hc/c on partitions, q on free):
  - host pre-transposes q_x/kv_x to [n, c, q] so projections need no on-chip transpose
  - attention computed as a^T [k, q]: QK via row-tiled (K=32) packed matmuls,
    tri_bias pre-accumulated into PSUM via identity-stationary f32r matmuls,
    mask_bias applied as the per-partition bias of the ACT exp
  - softmax denominator via column-tiled ones-matmul that also broadcasts the
    per-head sum across the head's 32 partitions; normalization folded into the
    sigmoid gate: o * sigmoid(g) / s == o / (s * (1 + exp(-g)))
  - output projection keeps q on partitions so the result DMAs out contiguously
Matmul dtypes: f32r for DMA-fed operands (projections, bias writes), bf16 for
the feedback path (QK, sums, AV, out-proj) — fp32 PSUM accumulation throughout.
"""
import sys

sys.path.insert(0, "/opt/trn_rl_repo")

import math

import numpy as np
import ml_dtypes

N_CORES = 8
B, N, Q, C = 1, 256, 256, 128
H, C_HID = 4, 32
ROWS = N // N_CORES  # rows per core

_cache = {}


def _build(mask_zero=True, repeats=1):
    import concourse.bass as bass
    import concourse.tile as tile
    from concourse import mybir, bacc

    f32 = mybir.dt.float32
    f32r = mybir.dt.float32r
    bf16 = mybir.dt.bfloat16
    Exp = mybir.ActivationFunctionType.Exp

    nc = bacc.Bacc("TRN2", target_bir_lowering=False, debug=False,
                   num_devices=N_CORES)

    G = 8  # rows per DMA batch
    NB = ROWS // G
    qxT = nc.dram_tensor("qxT", [NB, C, G * Q], f32r, kind="ExternalInput").ap()
    kxT = nc.dram_tensor("kxT", [NB, C, G * Q], f32r, kind="ExternalInput").ap()
    tri = nc.dram_tensor("tri", [128, 2 * H * Q], f32r, kind="ExternalInput").ap()
    wq = nc.dram_tensor("wq", [C, C], f32r, kind="ExternalInput").ap()
    wk = nc.dram_tensor("wk", [C, C], f32r, kind="ExternalInput").ap()
    wv = nc.dram_tensor("wv", [C, C], f32r, kind="ExternalInput").ap()
    wg = nc.dram_tensor("wg", [C, C], f32r, kind="ExternalInput").ap()
    wo = nc.dram_tensor("wo", [C, C], bf16, kind="ExternalInput").ap()
    eye = nc.dram_tensor("eye", [C, C], f32r, kind="ExternalInput").ap()
    ones32 = nc.dram_tensor("ones32", [128, 32], bf16, kind="ExternalInput").ap()
    if not mask_zero:
        maskd = nc.dram_tensor("maskd", [128, ROWS, 2], f32,
                               kind="ExternalInput").ap()
    out_d = nc.dram_tensor("out", [ROWS // 8, 128, 8 * 256], f32,
                           kind="ExternalOutput").ap()

    with tile.TileContext(nc) as tc:
        with tc.tile_pool(name="const", bufs=1) as cpool, \
             tc.tile_pool(name="xin", bufs=3) as xpool, \
             tc.tile_pool(name="projsb", bufs=2) as ppool, \
             tc.tile_pool(name="aexp", bufs=2) as epool, \
             tc.tile_pool(name="gate", bufs=2) as gpool, \
             tc.tile_pool(name="proj_ps", bufs=1, space="PSUM") as proj_pool, \
             tc.tile_pool(name="at_ps", bufs=1, space="PSUM") as at_pool, \
             tc.tile_pool(name="so_ps", bufs=1, space="PSUM") as so_pool:

            tri_sb = cpool.tile([128, 2 * H * Q], f32r)
            wq_sb = cpool.tile([C, C], f32r, tag="wq")
            wk_sb = cpool.tile([C, C], f32r, tag="wk")
            wv_sb = cpool.tile([C, C], f32r, tag="wv")
            wg_sb = cpool.tile([C, C], f32r, tag="wg")
            wo_sb = cpool.tile([C, C], bf16, tag="wo")
            eye_sb = cpool.tile([C, C], f32r, tag="eye")
            ones_sb = cpool.tile([128, 32], bf16, tag="ones")
            nc.sync.dma_start(out=tri_sb[:], in_=tri[:])
            nc.sync.dma_start(out=wq_sb[:], in_=wq[:])
            nc.sync.dma_start(out=wk_sb[:], in_=wk[:])
            nc.sync.dma_start(out=wv_sb[:], in_=wv[:])
            nc.sync.dma_start(out=wg_sb[:], in_=wg[:])
            nc.sync.dma_start(out=wo_sb[:], in_=wo[:])
            nc.sync.dma_start(out=eye_sb[:], in_=eye[:])
            nc.sync.dma_start(out=ones_sb[:], in_=ones32[:])
            if not mask_zero:
                mask_sb = cpool.tile([128, ROWS, 2], f32, tag="mask")
                nc.sync.dma_start(out=mask_sb[:], in_=maskd[:])

            for _rep in range(repeats):
             for b in range(ROWS // G):
              qxb_sb = xpool.tile([C, G * Q], f32r, tag="qx")
              kxb_sb = xpool.tile([C, G * Q], f32r, tag="kx")
              nc.sync.dma_start(out=qxb_sb[:], in_=qxT[b])
              nc.sync.dma_start(out=kxb_sb[:], in_=kxT[b])
              ost = gpool.tile([128, G * 256], f32, tag="ost")
              for r in range(G):
                n = b * G + r
                qx_sb = qxb_sb[:, r * Q:(r + 1) * Q]
                kx_sb = kxb_sb[:, r * Q:(r + 1) * Q]

                # proj mega psum tile: qT 0:256 | kT 256:512 | v 512:768 |
                # gT 768:1024 | oT 1024:1280   (3 banks)
                pp = proj_pool.tile([128, 1536], f32, tag="pp")
                qT_ps = pp[:, 0:256]
                kT_ps = pp[:, 256:512]
                v_ps = pp[:, 512:768]
                gT_ps = pp[:, 768:1024]
                oT_ps = pp[:, 1024:1280]

                # bank0 group: qT then kT; bank1 group: v(kc0,kc1) then gT
                nc.tensor.matmul(qT_ps, lhsT=wq_sb[:], rhs=qx_sb[:],
                                 start=True, stop=False, skip_group_check=True)
                nc.tensor.matmul(kT_ps, lhsT=wk_sb[:], rhs=kx_sb[:],
                                 start=False, stop=True, skip_group_check=True)
                for kc in range(2):
                    nc.tensor.matmul(v_ps[:, kc * 128:(kc + 1) * 128],
                                     lhsT=kx_sb[:, kc * 128:(kc + 1) * 128],
                                     rhs=wv_sb[:], start=(kc == 0), stop=False, skip_group_check=True)
                nc.tensor.matmul(gT_ps, lhsT=wg_sb[:], rhs=qx_sb[:],
                                 start=False, stop=True, skip_group_check=True)

                # aT mega tile [128, (kc, h, q)] (4 banks); tri_bias written
                # first (start=True), QK accumulates on top
                # each 2KB bank holds two 256-col slices (h even/odd pair):
                # one group per bank spanning [bias even, bias odd, QK even, QK odd]
                at = at_pool.tile([128, 2 * H * Q], f32, tag="at")
                for h in range(H):
                    for kc in range(2):
                        s = (h * 2 + kc) * Q
                        nc.tensor.matmul(at[:, s:s + Q], lhsT=eye_sb[:],
                                         rhs=tri_sb[:, s:s + Q],
                                         start=(kc == 0), stop=False, skip_group_check=True)

                # evacuate q/k/v projections in one copy (bf16)
                qkv_sb = ppool.tile([C, 768], bf16, tag="qkv")
                nc.vector.tensor_copy(out=qkv_sb[:], in_=pp[:, 0:768])
                qT_sb = qkv_sb[:, 0:256]
                kT_sb = qkv_sb[:, 256:512]
                v_sb = qkv_sb[:, 512:768]
                # exp(-g) for the sigmoid-gate fold (same ACT table as Exp)
                ag_sb = ppool.tile([C, Q], f32, tag="ag")
                nc.scalar.activation(ag_sb[:], gT_ps, Exp, scale=-1.0)

                # QK: row-tiled K=32 packed matmuls, accumulate onto tri bias
                for kc in range(2):
                    for h in range(H):
                        s = (h * 2 + kc) * Q
                        nc.tensor.matmul(
                            at[:, s:s + Q],
                            lhsT=kT_sb[32 * h:32 * (h + 1), kc * 128:(kc + 1) * 128],
                            rhs=qT_sb[32 * h:32 * (h + 1), :],
                            start=False, stop=(kc == 1),
                            tile_position=(32 * h, 0),
                            skip_group_check=True)

                # softmax numerator: exp over the whole mega tile
                aexp_sb = epool.tile([128, 2 * H * Q], bf16, tag="aexp")
                if mask_zero:
                    nc.scalar.activation(aexp_sb[:], at[:], Exp)
                else:
                    atv = at[:].rearrange("p (h k q) -> p h k q", h=H, k=2)
                    aev = aexp_sb[:].rearrange("p (h k q) -> p h k q", h=H, k=2)
                    for kc in range(2):
                        nc.scalar.activation(aev[:, :, kc, :], atv[:, :, kc, :],
                                             Exp, bias=mask_sb[:, n, kc])

                # denominator, broadcast across each head's 32 partitions via
                # all-ones stationary; col-tiled packed
                so_t = so_pool.tile([128, 512], f32, tag="so")
                so = so_t[:, 0:256]
                for kc in range(2):
                    for h in range(H):
                        s = (h * 2 + kc) * Q
                        nc.tensor.matmul(so[32 * h:32 * (h + 1), :],
                                         lhsT=ones_sb[:],
                                         rhs=aexp_sb[:, s:s + Q],
                                         start=(kc == 0), stop=(kc == 1),
                                         tile_position=(0, 32 * h),
                                         skip_group_check=True)
                # attention values: col-tiled packed, accumulate over kc
                for kc in range(2):
                    for h in range(H):
                        s = (h * 2 + kc) * Q
                        nc.tensor.matmul(
                            pp[32 * h:32 * (h + 1), 1024:1280],
                            lhsT=v_sb[:, kc * 128 + 32 * h:kc * 128 + 32 * (h + 1)],
                            rhs=aexp_sb[:, s:s + Q],
                            start=(kc == 0), stop=(kc == 1),
                            tile_position=(0, 32 * h),
                            skip_group_check=True)

                # gate+normalize: ge = 1/(s*(1+exp(-g))); o_f = oT * ge
                u2 = gpool.tile([C, Q], f32, tag="u2")
                ge = gpool.tile([C, Q], f32, tag="ge")
                of_sb = gpool.tile([C, Q], bf16, tag="of")
                nc.vector.scalar_tensor_tensor(
                    out=u2[:], in0=ag_sb[:], scalar=1.0, in1=so[:],
                    op0=mybir.AluOpType.add, op1=mybir.AluOpType.mult)
                nc.vector.reciprocal_approx_fast(out=ge[:], in_=u2[:])
                nc.vector.tensor_tensor(out=of_sb[:], in0=oT_ps, in1=ge[:],
                                        op=mybir.AluOpType.mult)

                # output projection: q on partitions, c on free -> direct DMA
                op_t = so_pool.tile([128, 512], f32, tag="so")
                op = op_t[:, 0:256]
                for qc in range(2):
                    nc.tensor.matmul(op[:, qc * 128:(qc + 1) * 128],
                                     lhsT=of_sb[:, qc * 128:(qc + 1) * 128],
                                     rhs=wo_sb[:], start=(qc == 0), stop=(qc == 1), skip_group_check=True)
                nc.vector.tensor_copy(out=ost[:, r * 256:(r + 1) * 256],
                                      in_=op[:])
              nc.sync.dma_start(out=out_d[b], in_=ost[:])
    nc.compile()
    return nc


def _host_prep(inputs):
    q_x = np.ascontiguousarray(inputs["q_x"], np.float32)[0]    # [N, Q, C]
    kv_x = np.ascontiguousarray(inputs["kv_x"], np.float32)[0]
    tri_b = np.asarray(inputs["tri_bias"], np.float32)[0, 0]    # [H, Q, K]
    mask_b = np.asarray(inputs["mask_bias"], np.float32)[0, :, 0, 0, :]  # [N, K]
    Wq = np.asarray(inputs["Wq"], np.float32) / math.sqrt(C_HID)
    Wk = np.asarray(inputs["Wk"], np.float32)
    Wv = np.asarray(inputs["Wv"], np.float32)
    Wg = np.asarray(inputs["Wg"], np.float32)
    Wo = np.asarray(inputs["Wo"], np.float32)

    # batched layout: [N/8, C, 8*Q]; arr[b, c, r*Q+q] = x[8b+r, q, c]
    def batch_T(x):
        return np.ascontiguousarray(
            x.reshape(N // 8, 8, Q, C).transpose(0, 3, 1, 2).reshape(N // 8, C, 8 * Q))
    qxT = batch_T(q_x)
    kxT = batch_T(kv_x)

    # tri layout: [128, (h, kc, q)]; tri[p, (h*2+kc)*Q + q] = tri_b[h, q, kc*128+p]
    tri_dev = np.empty((128, 2 * H * Q), np.float32)
    for h in range(H):
        for kc in range(2):
            s = (h * 2 + kc) * Q
            tri_dev[:, s:s + Q] = tri_b[h, :, kc * 128:(kc + 1) * 128].T

    shared = {
        "tri": tri_dev,
        "wq": Wq, "wk": Wk, "wv": Wv, "wg": Wg,
        "wo": Wo.astype(ml_dtypes.bfloat16),
        "eye": np.eye(C, dtype=np.float32),
        "ones32": np.ones((128, 32), ml_dtypes.bfloat16),
    }
    nb = ROWS // 8
    in_maps = []
    for c in range(N_CORES):
        b0 = c * nb
        in_maps.append({
            "qxT": np.ascontiguousarray(qxT[b0:b0 + nb]),
            "kxT": np.ascontiguousarray(kxT[b0:b0 + nb]),
            **shared,
        })
    return in_maps, mask_b


def kernel(**inputs):
    from concourse import bass_utils

    in_maps, mask_b = _host_prep(inputs)
    mask_zero = bool(np.all(mask_b == 0.0))
    if not mask_zero:
        # mask layout [128, rows, kc]: mask[p, n, kc] = mask_b[row, kc*128+p]
        for c in range(N_CORES):
            r0 = c * ROWS
            md = np.empty((128, ROWS, 2), np.float32)
            for kc in range(2):
                md[:, :, kc] = mask_b[r0:r0 + ROWS, kc * 128:(kc + 1) * 128].T
            in_maps[c]["maskd"] = md
    key = ("nc", mask_zero)
    if key not in _cache:
        _cache[key] = _build(mask_zero)
    nc = _cache[key]
    res = bass_utils.run_bass_kernel_spmd(nc, in_maps, list(range(N_CORES)))
    # device layout [NB, 128(qp), 8(r), 2(qc), 128(c)] -> [n, q, c]
    out = np.concatenate([res.results[c]["out"] for c in range(N_CORES)], axis=0)
    out = out.reshape(N // 8, 128, 8, 2, 128).transpose(0, 2, 3, 1, 4)
    return np.ascontiguousarray(out.reshape(B, N, Q, C))
